# revision 6
# baseline (speedup 1.0000x reference)
"""Causal self-attention kernel for Trainium2, 8 NeuronCores.

Problem: B=4, T=2048, C=1024, 16 heads, head_dim=64, fp32.
  q = x@Wq.T, k = x@Wk.T, v = x@Wv.T  (heads split)
  attn = softmax(causal(q@k.T/8)); out = (attn@v) @ Wo.T

Sharding: 8 cores = 4 batches x 2 head-groups (8 heads each).
Each core computes QKV projections for its (batch, head-group),
causal attention, and a partial output projection against its
512 columns of W_o; a pairwise ReduceScatter sums the two
head-group partials and leaves each core with half the T rows.
The host reassembles the full [4, 2048, 1024] output.

All matmuls run in float32r (TF32-like, ~1.5e-4 rel err, 4x faster
than fp32 on the PE). Softmax skips max-subtraction (scores are
~N(0,1); exp is safe in fp32) and gets the denominator for free by
augmenting v with a ones column (row 64 of the av matmul output).

Layouts (per core):
  xT   [C=8x128, T]        (PE-transposed on chip)
  qT/kT 4 tiles [128, T]   (head pair h=2m,2m+1 at partitions 0:64/64:128)
  v    16 tiles [128, 8, 65] (k-tile major; per head 64 dims + ones col)
  scores sT [k-tile 128, q 512] per (head, k-tile, q-chunk), both heads of
        a pair computed concurrently via PE row-tiling (K=64 each)
  avT  [65, q 512] accumulated in PSUM over k-tiles; row 64 = softmax denom
"""

import numpy as np
from contextlib import ExitStack

import concourse.bass as bass
import concourse.tile as tile
from concourse import bacc, mybir, bass_utils
from concourse.masks import make_identity

B, T, C = 4, 2048, 1024
NCORES = 8
NH = 8            # heads per core
HD = 64
S = NH * HD       # 512 = per-core qkv dim shard
TT = T // 128     # 16 T-tiles
CCH = C // 128    # 8 C-chunks
QC = T // 512     # 4 q-chunks of 512
F32 = mybir.dt.float32
F32R = mybir.dt.float32r
EXP = mybir.ActivationFunctionType.Exp
MULT = mybir.AluOpType.mult
RG = [[0, 1], [2, 3], [4, 5], [6, 7]]

_cache = {}


def _transpose_blocks(nc, ps_t, ident, src_ap, dst_ap, nblk):
    """PE-transpose nblk [128,128] blocks: src [128, nblk*128] -> dst.

    dst_ap must accept [128, 128] writes at block column b (caller passes a
    callable)."""
    for bkl in range(nblk):
        pst = ps_t.tile([128, 128], F32, name="pst", tag="pst")
        nc.tensor.transpose(pst[:], src_ap(bkl), ident)
        nc.vector.tensor_copy(dst_ap(bkl), pst[:])


def _build_kernel():
    nc = bacc.Bacc("TRN2", target_bir_lowering=False, debug=False,
                   num_devices=NCORES)
    x_d = nc.dram_tensor("x", [T, C], F32, kind="ExternalInput").ap()
    wq_d = nc.dram_tensor("wq", [S, C], F32, kind="ExternalInput").ap()
    wk_d = nc.dram_tensor("wk", [S, C], F32, kind="ExternalInput").ap()
    wv_d = nc.dram_tensor("wv", [S, C], F32, kind="ExternalInput").ap()
    wo_d = nc.dram_tensor("wo", [C, S], F32, kind="ExternalInput").ap()
    oh_d = nc.dram_tensor("o_half", [T // 2, C], F32,
                          kind="ExternalOutput").ap()

    with tile.TileContext(nc) as tc, ExitStack() as top:
        const = top.enter_context(tc.tile_pool(name="const", bufs=1))
        dram = top.enter_context(tc.tile_pool(name="dram", bufs=1,
                                              space="DRAM"))
        ident = const.tile([128, 128], F32, name="ident")
        make_identity(nc, ident[:])
        # tri[kk, u] = 1 if u >= kk else 0  (keep where q >= k on the diag)
        tri_f = const.tile([128, 128], F32, name="tri_f")
        nc.gpsimd.memset(tri_f[:], 1.0)
        nc.gpsimd.affine_select(
            out=tri_f[:], in_=tri_f[:], compare_op=mybir.AluOpType.is_ge,
            fill=0.0, base=0, pattern=[[1, 128]], channel_multiplier=-1)
        tri = const.tile([128, 128], F32R, name="tri")
        nc.vector.tensor_copy(tri[:], tri_f[:])
        ones8_f = const.tile([128, 8], F32, name="ones8_f")
        nc.gpsimd.memset(ones8_f[:], 1.0)

        obuf = dram.tile([T, C], F32, name="obuf")
        orec = dram.tile([T // 2, C], F32, name="orec")

        # persistent SBUF tensors
        persist = top.enter_context(tc.tile_pool(name="persist", bufs=1))
        qT = [persist.tile([128, T], F32R, name=f"qT{m}", tag=f"qT{m}")
              for m in range(4)]
        kT = [persist.tile([128, T], F32R, name=f"kT{m}", tag=f"kT{m}")
              for m in range(4)]
        vt = [persist.tile([128, NH, HD + 1], F32R, name=f"v{t}", tag=f"v{t}")
              for t in range(TT)]
        woT = persist.tile([128, 4, C], F32R, name="woT")

        with ExitStack() as ph12:
            ps_t = ph12.enter_context(
                tc.tile_pool(name="ps_t", bufs=4, space="PSUM"))
            ps_p = ph12.enter_context(
                tc.tile_pool(name="ps_p", bufs=4, space="PSUM"))
            wnat = ph12.enter_context(tc.tile_pool(name="wnat", bufs=2))
            wT_pool = ph12.enter_context(tc.tile_pool(name="wT_pool", bufs=1))
            xnat = ph12.enter_context(tc.tile_pool(name="xnat", bufs=2))
            xtn_pool = ph12.enter_context(tc.tile_pool(name="xtn", bufs=1))

            # ---- weights: load + transpose to fp32r ----
            wqT = wT_pool.tile([128, CCH, S], F32R, name="wqT")
            wkT = wT_pool.tile([128, CCH, S], F32R, name="wkT")
            wvT = wT_pool.tile([128, CCH, S], F32R, name="wvT")
            for w_d, wT in ((wq_d, wqT), (wk_d, wkT), (wv_d, wvT)):
                for m in range(4):
                    wn = wnat.tile([128, C], F32, name="wn", tag="wn")
                    nc.sync.dma_start(wn[:], w_d[m * 128:(m + 1) * 128, :])
                    _transpose_blocks(
                        nc, ps_t, ident[:],
                        lambda c: wn[:, c * 128:(c + 1) * 128],
                        lambda c: wT[:, c, m * 128:(m + 1) * 128], CCH)
            for t2 in range(8):
                wn = wnat.tile([128, S], F32, name="wn2", tag="wn2")
                nc.sync.dma_start(wn[:], wo_d[t2 * 128:(t2 + 1) * 128, :])
                _transpose_blocks(
                    nc, ps_t, ident[:],
                    lambda m: wn[:, m * 128:(m + 1) * 128],
                    lambda m: woT[:, m, t2 * 128:(t2 + 1) * 128], 4)

            # ---- x transpose + projections, streamed per 512-col chunk ----
            for n in range(QC):
                xtn = xtn_pool.tile([128, CCH, 512], F32R, name="xtn",
                                    tag="xtn")
                for tl in range(4):
                    t = 4 * n + tl
                    xn = xnat.tile([128, C], F32, name="xn", tag="xn")
                    nc.sync.dma_start(xn[:], x_d[t * 128:(t + 1) * 128, :])
                    _transpose_blocks(
                        nc, ps_t, ident[:],
                        lambda c: xn[:, c * 128:(c + 1) * 128],
                        lambda c, tl=tl: xtn[:, c, tl * 128:(tl + 1) * 128],
                        CCH)
                # qT / kT part-tiles for this T-chunk
                for wT, dstl in ((wqT, qT), (wkT, kT)):
                    for m in range(4):
                        psq = ps_p.tile([128, 512], F32, name="psq", tag="psq")
                        for c in range(CCH):
                            nc.tensor.matmul(
                                psq[:], wT[:, c, m * 128:(m + 1) * 128],
                                xtn[:, c, :],
                                start=(c == 0), stop=(c == CCH - 1))
                        nc.vector.tensor_copy(
                            dstl[m][:, n * 512:(n + 1) * 512], psq[:])
                # v for the 4 T-tiles of this chunk
                for tl in range(4):
                    t = 4 * n + tl
                    psv = ps_p.tile([128, 512], F32, name="psv", tag="psq")
                    for c in range(CCH):
                        nc.tensor.matmul(
                            psv[:], xtn[:, c, tl * 128:(tl + 1) * 128],
                            wvT[:, c, :],
                            start=(c == 0), stop=(c == CCH - 1))
                    nc.vector.tensor_copy(
                        vt[t][:, :, 0:64],
                        psv[:].rearrange("p (h d) -> p h d", h=NH))
                    nc.scalar.copy(vt[t][:, :, 64], ones8_f[:])

        # ---- attention + output projection ----
        with ExitStack() as ph34:
            ps_s = ph34.enter_context(
                tc.tile_pool(name="ps_s", bufs=2, space="PSUM"))
            ps_avA = ph34.enter_context(
                tc.tile_pool(name="ps_avA", bufs=1, space="PSUM"))
            ps_avB = ph34.enter_context(
                tc.tile_pool(name="ps_avB", bufs=1, space="PSUM"))
            ps_o = ph34.enter_context(
                tc.tile_pool(name="ps_o", bufs=2, space="PSUM"))
            p_pool = ph34.enter_context(tc.tile_pool(name="p_pool", bufs=3))
            rl_pool = ph34.enter_context(tc.tile_pool(name="rl_pool", bufs=4))
            rlb_pool = ph34.enter_context(
                tc.tile_pool(name="rlb_pool", bufs=4))
            o_pool = ph34.enter_context(tc.tile_pool(name="o_pool", bufs=2))
            avt_pool = ph34.enter_context(
                tc.tile_pool(name="avt_pool", bufs=1))
            avT = [avt_pool.tile([128, T], F32R, name=f"avT{m}",
                                 tag=f"avT{m}") for m in range(4)]

            def attention_chunk(i):
                nk = 4 * i + 4  # k-tiles 0..nk-1
                for m in range(4):  # head pairs
                    av_ps = [
                        ps_avA.tile([128, 512], F32, name="avA", tag="avA"),
                        ps_avB.tile([128, 512], F32, name="avB", tag="avB"),
                    ]
                    for j in range(nk):
                        r = j - 4 * i
                        lo = max(r, 0) * 128
                        w = 512 - lo
                        sps = ps_s.tile([128, 1024], F32, name="sps",
                                        tag="sps")
                        for s2 in range(2):
                            nc.tensor.matmul(
                                sps[:, s2 * 512:(s2 + 1) * 512],
                                kT[m][64 * s2:64 * s2 + 64,
                                      j * 128:(j + 1) * 128],
                                qT[m][64 * s2:64 * s2 + 64,
                                      i * 512:(i + 1) * 512],
                                start=True, stop=True)
                        pp = p_pool.tile([128, 1024], F32R, name="pp",
                                         tag="pp")
                        nc.scalar.activation(
                            pp[:].rearrange("p (s q) -> p s q", s=2)
                                [:, :, lo:512],
                            sps[:].rearrange("p (s q) -> p s q", s=2)
                                [:, :, lo:512],
                            EXP, scale=0.125)
                        if r >= 0:
                            for s2 in range(2):
                                blk = pp[:, s2 * 512 + lo:s2 * 512 + lo + 128]
                                nc.vector.tensor_tensor(blk, blk, tri[:],
                                                        op=MULT)
                        for s2 in range(2):
                            h = 2 * m + s2
                            nc.tensor.matmul(
                                av_ps[s2][0:65, lo:512],
                                vt[j][:, h, :],
                                pp[:, s2 * 512 + lo:(s2 + 1) * 512],
                                start=(j == 0), stop=(j == nk - 1))
                    for s2 in range(2):
                        rl = rl_pool.tile([1, 512], F32, name="rl", tag="rl")
                        nc.vector.reciprocal(rl[:], av_ps[s2][64:65, :])
                        rlb = rlb_pool.tile([64, 512], F32, name="rlb",
                                            tag="rlb")
                        nc.gpsimd.partition_broadcast(rlb[:], rl[:])
                        nc.vector.tensor_tensor(
                            avT[m][64 * s2:64 * s2 + 64,
                                   i * 512:(i + 1) * 512],
                            av_ps[s2][0:64, :], rlb[:], op=MULT)

            def oproj_chunk(i):
                for tl in range(4):
                    t = 4 * i + tl
                    osb = o_pool.tile([128, C], F32, name="osb", tag="osb")
                    for nh2 in range(2):
                        pso = ps_o.tile([128, 512], F32, name="pso",
                                        tag="pso")
                        for m in range(4):
                            nc.tensor.matmul(
                                pso[:],
                                avT[m][:, t * 128:(t + 1) * 128],
                                woT[:, m, nh2 * 512:(nh2 + 1) * 512],
                                start=(m == 0), stop=(m == 3))
                        nc.vector.tensor_copy(
                            osb[:, nh2 * 512:(nh2 + 1) * 512], pso[:])
                    nc.sync.dma_start(obuf[t * 128:(t + 1) * 128, :], osb[:])

            def reduce_chunk(i):
                nc.gpsimd.collective_compute(
                    "ReduceScatter", mybir.AluOpType.add,
                    replica_groups=RG,
                    ins=[obuf[512 * i:512 * (i + 1), :]],
                    outs=[orec[256 * i:256 * (i + 1), :]])
                nc.sync.dma_start(oh_d[256 * i:256 * (i + 1), :],
                                  orec[256 * i:256 * (i + 1), :])

            # interleave so each chunk's ReduceScatter overlaps the next
            # chunk's attention (the collective wait sits on gpsimd)
            attention_chunk(0)
            oproj_chunk(0)
            attention_chunk(1)
            reduce_chunk(0)
            oproj_chunk(1)
            attention_chunk(2)
            reduce_chunk(1)
            oproj_chunk(2)
            attention_chunk(3)
            reduce_chunk(2)
            oproj_chunk(3)
            reduce_chunk(3)

    nc.compile()
    return nc


def _get_nc():
    if "nc" not in _cache:
        _cache["nc"] = _build_kernel()
    return _cache["nc"]


def _in_maps(x, W_q, W_k, W_v, W_o):
    maps = []
    for core in range(NCORES):
        b, g = core // 2, core % 2
        sl = slice(g * S, (g + 1) * S)
        maps.append({
            "x": np.ascontiguousarray(x[b]),
            "wq": np.ascontiguousarray(W_q[sl]),
            "wk": np.ascontiguousarray(W_k[sl]),
            "wv": np.ascontiguousarray(W_v[sl]),
            "wo": np.ascontiguousarray(W_o[:, sl]),
        })
    return maps


def _assemble(results):
    out = np.empty((B, T, C), np.float32)
    for b in range(B):
        ev = results[2 * b]["o_half"]
        od = results[2 * b + 1]["o_half"]
        for i in range(QC):
            out[b, 512 * i:512 * i + 256] = ev[256 * i:256 * i + 256]
            out[b, 512 * i + 256:512 * (i + 1)] = od[256 * i:256 * i + 256]
    return out


def kernel(x, W_q, W_k, W_v, W_o):
    nc = _get_nc()
    res = bass_utils.run_bass_kernel_spmd(
        nc, _in_maps(x, W_q, W_k, W_v, W_o), core_ids=list(range(NCORES)))
    return _assemble(res.results)


def run_traced(x, W_q, W_k, W_v, W_o, **kwargs):
    """Dev helper: run with tracing; returns (output, BassKernelResults)."""
    nc = _get_nc()
    res = bass_utils.run_bass_kernel_spmd(
        nc, _in_maps(x, W_q, W_k, W_v, W_o), core_ids=list(range(NCORES)),
        trace=True, **kwargs)
    return _assemble(res.results), res


# revision 17
# speedup vs baseline: 261.1597x; 261.1597x over previous
"""Causal self-attention kernel for Trainium2, 8 NeuronCores.

Problem: B=4, T=2048, C=1024, 16 heads, head_dim=64, fp32.
  q = x@Wq.T, k = x@Wk.T, v = x@Wv.T  (heads split)
  attn = softmax(causal(q@k.T/8)); out = (attn@v) @ Wo.T

Sharding: 8 cores = 4 batches x 2 head-groups (8 heads each).
Each core computes QKV projections for its (batch, head-group),
causal attention, and a partial output projection against its
512 columns of W_o; a pairwise ReduceScatter sums the two
head-group partials and leaves each core with half the T rows.
The host reassembles the full [4, 2048, 1024] output and feeds
pre-transposed x / weight shards so no on-chip transposes are needed.

All matmuls run in float32r (TF32-like, ~2e-4 rel err, 4x faster than
fp32 on the PE). Softmax skips max-subtraction (scores are ~N(0,1);
exp is safe in fp32) and gets the denominator for free by augmenting
v with a ones column (row 64 of the av matmul output).

Per-core layouts:
  xT   [C, T] streamed per 512-col chunk as [128, 8, 512]
  qT/kT [128, 4, T]         (head pair 2m,2m+1 at partitions 0:64/64:128
                             of plane m)
  v    [128, 16, 8, 65]     (k-tile, head, 64 dims + ones col)
  scores sT [k-tile 128, q 512] per (head, k-tile, q-chunk); both heads
        of a pair run concurrently via PE row-tiling (K=64 each) into
        one 2-bank PSUM tile, one exp covers both
  avT  [65, q 512] accumulated in PSUM over k-tiles; row 64 = denom
"""

import numpy as np
from contextlib import ExitStack

import concourse.bass as bass
import concourse.tile as tile
from concourse import bacc, mybir, bass_utils

B, T, C = 4, 2048, 1024
NCORES = 8
NH = 8            # heads per core
HD = 64
S = NH * HD       # 512 = per-core qkv dim shard
TT = T // 128     # 16 T-tiles
CCH = C // 128    # 8 C-chunks
QC = T // 512     # 4 q-chunks of 512
F32 = mybir.dt.float32
F32R = mybir.dt.float32r
EXP = mybir.ActivationFunctionType.Exp
MULT = mybir.AluOpType.mult
RG = [[0, 1], [2, 3], [4, 5], [6, 7]]

_cache = {}


def _build_kernel(collective=True, repeat=1):
    nc = bacc.Bacc("TRN2", target_bir_lowering=False, debug=False,
                   num_devices=NCORES)
    # all matmul operands arrive pre-transposed from the host, fp32r
    xT_d = nc.dram_tensor("xT", [C, T], F32R, kind="ExternalInput").ap()
    wqT_d = nc.dram_tensor("wqT", [C, S], F32R, kind="ExternalInput").ap()
    wkT_d = nc.dram_tensor("wkT", [C, S], F32R, kind="ExternalInput").ap()
    wvT_d = nc.dram_tensor("wvT", [C, S], F32R, kind="ExternalInput").ap()
    woT_d = nc.dram_tensor("woT", [S, C], F32R, kind="ExternalInput").ap()
    oh_d = nc.dram_tensor("o_half", [T // 2, C], F32,
                          kind="ExternalOutput").ap()

    with tile.TileContext(nc) as tc, ExitStack() as top:
        const = top.enter_context(tc.tile_pool(name="const", bufs=1))
        dram = top.enter_context(tc.tile_pool(name="dram", bufs=1,
                                              space="DRAM"))
        # tri[kk, u] = 1 if u >= kk else 0 (keep where q >= k on the diag)
        tri_f = const.tile([128, 128], F32, name="tri_f")
        nc.gpsimd.memset(tri_f[:], 1.0)
        nc.gpsimd.affine_select(
            out=tri_f[:], in_=tri_f[:], compare_op=mybir.AluOpType.is_ge,
            fill=0.0, base=0, pattern=[[1, 128]], channel_multiplier=-1)
        tri = const.tile([128, 128], F32R, name="tri")
        nc.vector.tensor_copy(tri[:], tri_f[:])
        ones16_f = const.tile([128, 16], F32, name="ones16_f")
        nc.gpsimd.memset(ones16_f[:], 1.0)

        obuf = dram.tile([T, C], F32, name="obuf")
        orec = dram.tile([T // 2, C], F32, name="orec")

        persist = top.enter_context(tc.tile_pool(name="persist", bufs=1))
        wqT = persist.tile([128, CCH, S], F32R, name="wqT")
        wkT = persist.tile([128, CCH, S], F32R, name="wkT")
        wvT = persist.tile([128, CCH, S], F32R, name="wvT")
        woT = persist.tile([128, 4, C], F32R, name="woT")
        kT = persist.tile([128, 4, T], F32R, name="kT")
        vt = persist.tile([128, TT, NH, HD + 1], F32R, name="vt")

        with ExitStack() as body:
            ps_pool = body.enter_context(
                tc.tile_pool(name="ps_pool", bufs=2, space="PSUM"))
            ps_av = body.enter_context(
                tc.tile_pool(name="ps_av", bufs=2, space="PSUM"))
            xtn_pool = body.enter_context(tc.tile_pool(name="xtn", bufs=1))
            qt_pool = body.enter_context(tc.tile_pool(name="qt_pool", bufs=2))
            avt_pool = body.enter_context(
                tc.tile_pool(name="avt_pool", bufs=2))
            p_pool = body.enter_context(tc.tile_pool(name="p_pool", bufs=3))
            rl_pool = body.enter_context(tc.tile_pool(name="rl_pool", bufs=2))
            rlb_pool = body.enter_context(
                tc.tile_pool(name="rlb_pool", bufs=2))
            o_pool = body.enter_context(tc.tile_pool(name="o_pool", bufs=2))

            xT_r = xT_d.rearrange("(c p) t -> p c t", p=128)

            def proj_chunk(n):
                xtn = xtn_pool.tile([128, CCH, 512], F32R, name="xtn",
                                    tag="xtn")
                for c in range(CCH):
                    nc.sync.dma_start(xtn[:, c],
                                      xT_r[:, c, n * 512:(n + 1) * 512])
                if n == 0:
                    wq_r = wqT_d.rearrange("(c p) s -> p c s", p=128)
                    wk_r = wkT_d.rearrange("(c p) s -> p c s", p=128)
                    wv_r = wvT_d.rearrange("(c p) s -> p c s", p=128)
                    for m in range(4):
                        nc.scalar.dma_start(
                            wqT[:, :, m * 128:(m + 1) * 128],
                            wq_r[:, :, m * 128:(m + 1) * 128])
                    for m in range(4):
                        nc.scalar.dma_start(
                            wkT[:, :, m * 128:(m + 1) * 128],
                            wk_r[:, :, m * 128:(m + 1) * 128])
                    nc.scalar.dma_start(wvT[:], wv_r)
                qTc = qt_pool.tile([128, 4, 512], F32R, name="qTc", tag="qTc")
                for wT, dst in ((wqT, qTc), (wkT, kT)):
                    for mp in range(2):  # plane pairs (0,1), (2,3)
                        ps = ps_pool.tile([128, 1024], F32, name="ps",
                                          tag="ps")
                        for half in range(2):
                            m = 2 * mp + half
                            for c in range(CCH):
                                nc.tensor.matmul(
                                    ps[:, half * 512:(half + 1) * 512],
                                    wT[:, c, m * 128:(m + 1) * 128],
                                    xtn[:, c, :],
                                    start=(c == 0), stop=(c == CCH - 1))
                        if dst is qTc:
                            nc.vector.tensor_copy(
                                qTc[:, 2 * mp:2 * mp + 2, :],
                                ps[:].rearrange("p (a q) -> p a q", a=2))
                        else:
                            nc.vector.tensor_copy(
                                dst[:, 2 * mp:2 * mp + 2,
                                    n * 512:(n + 1) * 512],
                                ps[:].rearrange("p (a q) -> p a q", a=2))
                for tp in range(2):  # T-tile pairs within the chunk
                    ps = ps_pool.tile([128, 1024], F32, name="ps", tag="ps")
                    for half in range(2):
                        tl = 2 * tp + half
                        for c in range(CCH):
                            nc.tensor.matmul(
                                ps[:, half * 512:(half + 1) * 512],
                                xtn[:, c, tl * 128:(tl + 1) * 128],
                                wvT[:, c, :],
                                start=(c == 0), stop=(c == CCH - 1))
                    t0 = 4 * n + 2 * tp
                    nc.vector.tensor_copy(
                        vt[:, t0:t0 + 2, :, 0:64],
                        ps[:].rearrange("p (a h d) -> p a h d", a=2, h=NH))
                    nc.scalar.copy(
                        vt[:, t0:t0 + 2, :, 64],
                        ones16_f[:].rearrange("p (a h) -> p a h", a=2))
                return qTc

            def attention_chunk(i, qTc):
                nk = 4 * i + 4  # k-tiles 0..nk-1
                avc = avt_pool.tile([128, 4, 512], F32R, name="avc",
                                    tag="avc")
                for m in range(4):  # head pairs
                    avp = ps_av.tile([128, 1024], F32, name="avp", tag="avp")
                    av_ps = [avp[:, 0:512], avp[:, 512:1024]]
                    for j in range(nk):
                        r = j - 4 * i
                        lo = max(r, 0) * 128
                        sps = ps_pool.tile([128, 1024], F32, name="sps",
                                           tag="ps")
                        for s2 in range(2):
                            nc.tensor.matmul(
                                sps[:, s2 * 512:(s2 + 1) * 512],
                                kT[64 * s2:64 * s2 + 64, m,
                                   j * 128:(j + 1) * 128],
                                qTc[64 * s2:64 * s2 + 64, m, :],
                                start=True, stop=True)
                        pp = p_pool.tile([128, 1024], F32R, name="pp",
                                         tag="pp")
                        nc.scalar.activation(
                            pp[:].rearrange("p (s q) -> p s q", s=2)
                                [:, :, lo:512],
                            sps[:].rearrange("p (s q) -> p s q", s=2)
                                [:, :, lo:512],
                            EXP, scale=0.125)
                        if r >= 0:
                            for s2 in range(2):
                                blk = pp[:, s2 * 512 + lo:s2 * 512 + lo + 128]
                                nc.vector.tensor_tensor(blk, blk, tri[:],
                                                        op=MULT)
                        for s2 in range(2):
                            h = 2 * m + s2
                            nc.tensor.matmul(
                                av_ps[s2][0:65, lo:512],
                                vt[:, j, h, :],
                                pp[:, s2 * 512 + lo:(s2 + 1) * 512],
                                start=(j == 0), stop=(j == nk - 1))
                    for s2 in range(2):
                        rl = rl_pool.tile([1, 512], F32, name="rl", tag="rl")
                        nc.vector.reciprocal(rl[:], av_ps[s2][64:65, :])
                        rlb = rlb_pool.tile([64, 512], F32, name="rlb",
                                            tag="rlb")
                        nc.gpsimd.partition_broadcast(rlb[:], rl[:])
                        nc.vector.tensor_tensor(
                            avc[64 * s2:64 * s2 + 64, m, :],
                            av_ps[s2][0:64, :], rlb[:], op=MULT)
                return avc

            def oproj_chunk(i, avc):
                for tl in range(4):
                    t = 4 * i + tl
                    pso = ps_pool.tile([128, 1024], F32, name="pso", tag="ps")
                    for nh2 in range(2):
                        for m in range(4):
                            nc.tensor.matmul(
                                pso[:, nh2 * 512:(nh2 + 1) * 512],
                                avc[:, m, tl * 128:(tl + 1) * 128],
                                woT[:, m, nh2 * 512:(nh2 + 1) * 512],
                                start=(m == 0), stop=(m == 3))
                    osb = o_pool.tile([128, C], F32, name="osb", tag="osb")
                    nc.vector.tensor_copy(osb[:], pso[:])
                    nc.sync.dma_start(obuf[t * 128:(t + 1) * 128, :], osb[:])

            def reduce_chunk(i):
                if not collective:
                    nc.sync.dma_start(oh_d[256 * i:256 * (i + 1), :],
                                      obuf[256 * i:256 * (i + 1), :])
                    return
                nc.gpsimd.collective_compute(
                    "ReduceScatter", mybir.AluOpType.add,
                    replica_groups=RG,
                    ins=[obuf[512 * i:512 * (i + 1), :]],
                    outs=[orec[256 * i:256 * (i + 1), :]])
                nc.sync.dma_start(oh_d[256 * i:256 * (i + 1), :],
                                  orec[256 * i:256 * (i + 1), :])

            for _it in range(repeat):
                q0 = proj_chunk(0)
                if _it == 0:
                    nc.scalar.dma_start(
                        woT[:], woT_d.rearrange("(m p) c -> p m c", p=128))
                av0 = attention_chunk(0, q0)
                q1 = proj_chunk(1)
                av1 = attention_chunk(1, q1)
                oproj_chunk(0, av0)
                q2 = proj_chunk(2)
                av2 = attention_chunk(2, q2)
                oproj_chunk(1, av1)
                reduce_chunk(0)
                q3 = proj_chunk(3)
                av3 = attention_chunk(3, q3)
                oproj_chunk(2, av2)
                reduce_chunk(1)
                oproj_chunk(3, av3)
                reduce_chunk(2)
                reduce_chunk(3)

    nc.compile()
    return nc


def _get_nc():
    if "nc" not in _cache:
        _cache["nc"] = _build_kernel()
    return _cache["nc"]


def _in_maps(x, W_q, W_k, W_v, W_o):
    x = np.asarray(x, dtype=np.float32)
    W_q = np.asarray(W_q, dtype=np.float32)
    W_k = np.asarray(W_k, dtype=np.float32)
    W_v = np.asarray(W_v, dtype=np.float32)
    W_o = np.asarray(W_o, dtype=np.float32)
    maps = []
    for core in range(NCORES):
        b, g = core // 2, core % 2
        sl = slice(g * S, (g + 1) * S)
        maps.append({
            "xT": np.ascontiguousarray(x[b].T),
            "wqT": np.ascontiguousarray(W_q[sl].T),
            "wkT": np.ascontiguousarray(W_k[sl].T),
            "wvT": np.ascontiguousarray(W_v[sl].T),
            "woT": np.ascontiguousarray(W_o[:, sl].T),
        })
    return maps


def _assemble(results):
    out = np.empty((B, T, C), np.float32)
    for b in range(B):
        ev = results[2 * b]["o_half"]
        od = results[2 * b + 1]["o_half"]
        for i in range(QC):
            out[b, 512 * i:512 * i + 256] = ev[256 * i:256 * i + 256]
            out[b, 512 * i + 256:512 * (i + 1)] = od[256 * i:256 * i + 256]
    return out


def kernel(x, W_q, W_k, W_v, W_o):
    nc = _get_nc()
    res = bass_utils.run_bass_kernel_spmd(
        nc, _in_maps(x, W_q, W_k, W_v, W_o), core_ids=list(range(NCORES)))
    return _assemble(res.results)
